# revision 11
# baseline (speedup 1.0000x reference)
"""Bert4KG decoder (B=4, T=512, S=512, D=1024, H=16, F=4096, L=2) on 8 TRN2
NeuronCores.

Sharding: 8-way data-parallel over (batch, sequence-half) — core c owns the
256 query tokens [h*256, h*256+256) of sample b, where b = c//2, h = c%2.
Everything in the model is per-token except self-attention K/V, which is
exchanged between the two cores of a sample via a pairwise AllGather.
Cross-attention K/V sources are kernel inputs, so each core computes the
full-S K/V for its sample locally (no communication).

On-device layout: canonical activation x is [tokens(partitions), D(free)]
fp32 (LayerNorm- and softmax-friendly); matmul operands are bf16 with fp32
PSUM accumulation. x^T / Q^T / K^T come from PE transposes or directly from
the projections. Softmax skips the max-subtraction (scores are O(1) at this
model's init scale; masked entries are exactly -1e20 -> exp == 0). The 1/sum
normalization is folded into the w -> bf16 cast pass.
"""
import contextlib
import json

import numpy as np
import ml_dtypes

import concourse.bass as bass
import concourse.mybir as mybir
import concourse.tile as tile
from concourse.bass_utils import run_bass_kernel_spmd
from concourse.masks import make_identity
from concourse.vector_clock import ScopedClock, VectorClock

F32 = mybir.dt.float32
BF16 = mybir.dt.bfloat16
AF = mybir.ActivationFunctionType
ALU = mybir.AluOpType

B, T, S, D, H, F, L = 4, 512, 512, 1024, 16, 4096, 2
DH = D // H           # 64
TC = 256              # tokens per core
NEG = -1e20
EPS = 1e-5
N_CORES = 8
RG = [[0, 1], [2, 3], [4, 5], [6, 7]]


# ---------------------------------------------------------------------------
# Environment workarounds: the walrus build here rejects instructions with
# more than one semaphore wait. (a) Replace the TileContext exit drain with
# single-wait NOPs on the SP queue. (b) Post-pass the BIR JSON to hoist extra
# waits onto single-wait EventSemaphore instructions inserted just before the
# owning instruction (same engine => same program order => same semantics).
# ---------------------------------------------------------------------------

def _patched_drain_and_barrier(self, tick_clock, wait_clock):
    nc = self.nc
    gc = tick_clock.global_clock
    n = len(gc)
    for p in range(n):
        if gc[p]:
            vc = VectorClock([gc[p] if q == p else 0 for q in range(n)])
            nop = nc.sync.nop(nofuse=True, hint=f"drain_wait_p{p}")
            wait_clock.add_sem_waits(nop.ins, ScopedClock({None: vc}))
    nc.sync.drain()
    nc.all_engine_barrier()
    assert self.sems is not None
    popped = nc._tile_sem_poison_stack.pop()
    assert popped is self._sem_poison
    nc.clear_and_free_semaphores(list(self.sems.allocated().values()))
    nc.all_engine_barrier()


_orig_to_json_bytes = bass.Bass.to_json_bytes


def _split_waits(m):
    uid = 0
    changed = False
    for f in m.get("functions", []):
        for blk in f.get("blocks", []):
            insts = blk.get("instructions")
            if not insts:
                continue
            out = []
            for ins in insts:
                si = ins.get("sync_info")
                ow = (si or {}).get("on_wait") or []
                if len(ow) > 1:
                    for w in ow[:-1]:
                        uid += 1
                        out.append({
                            "debug": ins.get("debug", 0),
                            "engine": ins["engine"],
                            "ins": [], "outs": [],
                            "name": f"{ins['name']}-hw{uid}",
                            "opcode": "EventSemaphore",
                            "sync_info": {"on_wait": [w]},
                        })
                    si["on_wait"] = ow[-1:]
                    changed = True
                out.append(ins)
            blk["instructions"] = out
    return changed


def _patched_to_json_bytes(self):
    raw = _orig_to_json_bytes(self)
    m = json.loads(raw)
    if _split_waits(m):
        raw = json.dumps(m).encode()
    return raw


tile.TileContext._drain_and_barrier = _patched_drain_and_barrier
bass.Bass.to_json_bytes = _patched_to_json_bytes


def _install_ntff_hook():
    """Register the NTFF profile hook the image's antenv lacks (trace=True
    only; harmless if the boot package is unavailable)."""
    import sys as _sys
    import types as _types
    if "antenv.axon_hooks" in _sys.modules:
        return
    try:
        _sys.path.insert(0, "/root/.axon_site")
        from trn_agent_boot.trn_boot import _ntff_profile_via_ctypes
        hook = [_ntff_profile_via_ctypes("/opt/axon/libaxon_pjrt.so")]
    except Exception:
        return
    mod = _types.ModuleType("antenv.axon_hooks")
    mod.get_axon_ntff_profile_hook = lambda: hook[0]
    mod.set_axon_ntff_profile_hook = lambda h: hook.__setitem__(0, h)
    _sys.modules["antenv.axon_hooks"] = mod


_install_ntff_hook()


# ---------------------------------------------------------------------------
# Kernel build
# ---------------------------------------------------------------------------

def _build_nc():
    nc = bass.Bass("TRN2", target_bir_lowering=False, debug=False,
                   num_devices=N_CORES)
    P = {}
    P["x_emb"] = nc.declare_dram_parameter("x_emb", [TC, D], F32, isOutput=False)
    P["pos"] = nc.declare_dram_parameter("pos", [TC, D], F32, isOutput=False)
    P["causal"] = nc.declare_dram_parameter("causal", [2, 128, S], F32, isOutput=False)
    P["smask"] = nc.declare_dram_parameter("smask", [4, 128, S], F32, isOutput=False)
    P["kvT"] = nc.declare_dram_parameter("kvT", [4, D, S], BF16, isOutput=False)
    P["wq"] = nc.declare_dram_parameter("wq", [L, 5, D, D], BF16, isOutput=False)
    P["wk"] = nc.declare_dram_parameter("wk", [L, 5, D, D], BF16, isOutput=False)
    P["wv"] = nc.declare_dram_parameter("wv", [L, 5, D, D], BF16, isOutput=False)
    P["wo"] = nc.declare_dram_parameter("wo", [L, 5, D, D], BF16, isOutput=False)
    P["bqs"] = nc.declare_dram_parameter("bqs", [L, 5, D], F32, isOutput=False)
    P["bk"] = nc.declare_dram_parameter("bk", [L, 5, D], F32, isOutput=False)
    P["bv"] = nc.declare_dram_parameter("bv", [L, 5, D], BF16, isOutput=False)
    P["bo"] = nc.declare_dram_parameter("bo", [L, 5, D], BF16, isOutput=False)
    P["w1"] = nc.declare_dram_parameter("w1", [L, D, F], BF16, isOutput=False)
    P["b1"] = nc.declare_dram_parameter("b1", [L, F], F32, isOutput=False)
    P["w2"] = nc.declare_dram_parameter("w2", [L, F, D], BF16, isOutput=False)
    P["b2"] = nc.declare_dram_parameter("b2", [L, D], BF16, isOutput=False)
    P["xout"] = nc.declare_dram_parameter("xout", [TC, D], F32, isOutput=True)

    with tile.TileContext(nc) as tc:
        _emit(nc, tc, P)
    return nc


def _emit(nc, tc, P):
    ctx = contextlib.ExitStack()
    with ctx:
        ec = ctx.enter_context
        const = ec(tc.tile_pool(name="const", bufs=1))
        xp = ec(tc.tile_pool(name="xp", bufs=8))        # x f32 [128,1024]
        xtp = ec(tc.tile_pool(name="xtp", bufs=10))     # xT bf16 [128,256]
        qp = ec(tc.tile_pool(name="qp", bufs=9))        # qT bf16 [128,256]
        kvp = ec(tc.tile_pool(name="kvp", bufs=10))     # kvT in bf16 [128,512]
        ktp = ec(tc.tile_pool(name="ktp", bufs=10))     # kT bf16 [128,512]
        vp = ec(tc.tile_pool(name="vp", bufs=5))        # V bf16 [128,1024]
        ap_ = ec(tc.tile_pool(name="ap", bufs=9))       # aT bf16 [128,256]
        wp = ec(tc.tile_pool(name="wp", bufs=12))       # W row-blocks bf16 [128,1024]
        wsp = ec(tc.tile_pool(name="wsp", bufs=6))      # streamed W tiles [128,1024]
        w1p = ec(tc.tile_pool(name="w1p", bufs=18))     # W1 col tiles [128,128]
        htp = ec(tc.tile_pool(name="htp", bufs=34))     # hT bf16 [128,256]
        sp = ec(tc.tile_pool(name="sp", bufs=3))        # softmax f32/bf16 [128,512]
        wtp = ec(tc.tile_pool(name="wtp", bufs=12))     # wT bf16 [128,256]
        stat = ec(tc.tile_pool(name="stat", bufs=8))    # [128,1]
        scrap = ec(tc.tile_pool(name="scrap", bufs=2))  # LN square scratch f32
        biasp = ec(tc.tile_pool(name="biasp", bufs=12))
        dram = ec(tc.tile_pool(name="dram", bufs=2, space="DRAM"))
        # PSUM: per-tag slots, total banks must stay <= 8:
        psA = ec(tc.tile_pool(name="psA", bufs=3, space="PSUM"))  # pp [128,512]
        psB = ec(tc.tile_pool(name="psB", bufs=2, space="PSUM"))  # scores [128,512]
        psC = ec(tc.tile_pool(name="psC", bufs=1, space="PSUM"))  # x transposes
        psD = ec(tc.tile_pool(name="psD", bufs=2, space="PSUM"))  # w transposes

        ident32 = const.tile([128, 128], F32)
        make_identity(nc, ident32[:])
        ident16 = const.tile([128, 128], BF16)
        make_identity(nc, ident16[:])
        ones1 = const.tile([1, 128], BF16)
        nc.gpsimd.memset(ones1[:], 1.0)
        eps_col = const.tile([128, 1], F32)
        nc.gpsimd.memset(eps_col[:], EPS)

        causal_sb = [const.tile([128, S], F32, name=f"causal{qc}") for qc in range(2)]
        for qc in range(2):
            nc.sync.dma_start(out=causal_sb[qc][:], in_=P["causal"][qc])
        smask_sb = [const.tile([128, S], F32, name=f"smask{s}") for s in range(4)]
        for s in range(4):
            nc.sync.dma_start(out=smask_sb[s][:], in_=P["smask"][s])

        # ---------------- helpers ----------------
        def ln_into(x_new, accums, tag):
            out_tiles = []
            for t in range(2):
                if len(accums[t]) == 2:
                    s1 = stat.tile([128, 1], F32, name=f"s1_{tag}_{t}", tag="s1")
                    nc.vector.tensor_add(s1[:], accums[t][0][:], accums[t][1][:])
                else:
                    s1 = accums[t][0]
                sq = scrap.tile([128, D], F32, name=f"sq_{tag}_{t}", tag="scrap")
                s2 = stat.tile([128, 1], F32, name=f"s2_{tag}_{t}", tag="s2")
                nc.scalar.activation(sq[:], x_new[t][:], AF.Square, accum_out=s2[:])
                m = stat.tile([128, 1], F32, name=f"m_{tag}_{t}", tag="m")
                nc.scalar.mul(m[:], s1[:], 1.0 / D)
                m2 = stat.tile([128, 1], F32, name=f"m2_{tag}_{t}", tag="m2")
                nc.vector.tensor_mul(m2[:], m[:], m[:])
                var = stat.tile([128, 1], F32, name=f"var_{tag}_{t}", tag="var")
                nc.vector.scalar_tensor_tensor(
                    out=var[:], in0=s2[:], scalar=1.0 / D, in1=m2[:],
                    op0=ALU.mult, op1=ALU.subtract)
                sd = stat.tile([128, 1], F32, name=f"sd_{tag}_{t}", tag="sd")
                nc.scalar.activation(sd[:], var[:], AF.Sqrt, bias=eps_col[:])
                rstd = stat.tile([128, 1], F32, name=f"rstd_{tag}_{t}", tag="rstd")
                nc.vector.reciprocal(rstd[:], sd[:])
                nm = stat.tile([128, 1], F32, name=f"nm_{tag}_{t}", tag="nm")
                nc.vector.scalar_tensor_tensor(
                    out=nm[:], in0=m[:], scalar=-1.0, in1=rstd[:],
                    op0=ALU.mult, op1=ALU.mult)
                xo = xp.tile([128, D], F32, name=f"x_{tag}_{t}", tag="x")
                nc.scalar.activation(xo[:], x_new[t][:], AF.Identity,
                                     bias=nm[:], scale=rstd[:])
                out_tiles.append(xo)
            return out_tiles

        def transpose_x(x_tiles, tag):
            xT = [xtp.tile([128, TC], BF16, name=f"xT_{tag}_{d}", tag="xT")
                  for d in range(8)]
            for t in range(2):
                for d in range(8):
                    tp = psC.tile([128, 128], F32, name=f"tp_{tag}_{t}_{d}", tag="pt")
                    nc.tensor.transpose(tp[:], x_tiles[t][:, d * 128:(d + 1) * 128],
                                        ident32[:])
                    nc.scalar.copy(xT[d][:, t * 128:(t + 1) * 128], tp[:])
            return xT

        def load_w_rows(w_ap, tag):
            tiles = []
            for kt in range(8):
                wt = wp.tile([128, D], BF16, name=f"w_{tag}_{kt}", tag="w")
                nc.sync.dma_start(out=wt[:], in_=w_ap[kt * 128:(kt + 1) * 128, :])
                tiles.append(wt)
            return tiles

        def bias_col(b_ap, oc, tag):
            bc = biasp.tile([128, 1], F32, name=f"bc_{tag}_{oc}", tag="bcol")
            nc.sync.dma_start(out=bc[:], in_=b_ap[oc * 128:(oc + 1) * 128]
                              .rearrange("(p o) -> p o", o=1))
            return bc

        def bias_row(b_ap, o2, tag):
            br = biasp.tile([1, 512], BF16, name=f"br_{tag}_{o2}", tag="brow")
            nc.sync.dma_start(out=br[:], in_=b_ap[o2 * 512:(o2 + 1) * 512]
                              .rearrange("(o n) -> o n", o=1))
            return br

        def proj_T(w_tiles, rhs_tiles, n_free, b_ap, tag, scale=1.0,
                   pool=None, width=None, ptag="ktp"):
            """out^T[d2, n] = W^T @ rhs (+ per-partition bias): 8 tiles."""
            pool = pool or ktp
            width = width or S
            outs = []
            for oc in range(8):
                pp = psA.tile([128, 512], F32, name=f"pj_{tag}_{oc}", tag="pp")
                for kt in range(8):
                    nc.tensor.matmul(pp[:, :n_free],
                                     w_tiles[kt][:, oc * 128:(oc + 1) * 128],
                                     rhs_tiles[kt][:, :n_free],
                                     start=(kt == 0), stop=(kt == 7))
                ot = pool.tile([128, width], BF16, name=f"pjo_{tag}_{oc}",
                               tag=ptag)
                bc = bias_col(b_ap, oc, tag)
                nc.scalar.activation(ot[:, :n_free], pp[:, :n_free], AF.Identity,
                                     bias=bc[:], scale=scale)
                outs.append(ot)
            return outs

        def proj_tok(w_tiles, kvT_tiles, n_tok, b_row_ap, tag):
            """V[tok, ch] = kv @ Wv + bv."""
            outs = []
            for t in range(n_tok // 128):
                vt = vp.tile([128, D], BF16, name=f"v_{tag}_{t}", tag="V")
                for o2 in range(2):
                    pp = psA.tile([128, 512], F32, name=f"pv_{tag}_{t}_{o2}",
                                  tag="pp")
                    for kt in range(8):
                        nc.tensor.matmul(pp[:],
                                         kvT_tiles[kt][:, t * 128:(t + 1) * 128],
                                         w_tiles[kt][:, o2 * 512:(o2 + 1) * 512],
                                         start=(kt == 0), stop=False)
                    br = bias_row(b_row_ap, o2, f"{tag}_{t}")
                    nc.tensor.matmul(pp[:], ones1[:], br[:], start=False, stop=True)
                    nc.scalar.copy(vt[:, o2 * 512:(o2 + 1) * 512], pp[:])
                outs.append(vt)
            return outs

        def attention(l, i, xT, kT, V, mask_for_qc, x_cur, tag):
            qT = proj_T(load_w_rows(P["wq"][l, i], f"wq_{tag}"), xT, TC,
                        P["bqs"][l, i], f"q_{tag}", scale=0.125, pool=qp,
                        width=TC, ptag="qp")
            aT = []
            for hp in range(8):
                wT = [[wtp.tile([128, TC], BF16,
                                name=f"wT_{tag}_{hp}_{sub}_{kt}", tag="wT")
                       for kt in range(4)] for sub in range(2)]
                for sub in range(2):
                    hd = hp * 2 + sub
                    oc, sl = hd // 2, slice((hd % 2) * 64, (hd % 2) * 64 + 64)
                    for qc in range(2):
                        spp = psB.tile([128, S], F32,
                                       name=f"s_{tag}_{hd}_{qc}", tag="spp")
                        nc.tensor.matmul(spp[:],
                                         qT[oc][sl, qc * 128:(qc + 1) * 128],
                                         kT[oc][sl, :], start=True, stop=True)
                        ssb = sp.tile([128, S], F32, name=f"ss_{tag}_{hd}_{qc}",
                                      tag="ssb")
                        nc.vector.scalar_tensor_tensor(
                            out=ssb[:], in0=spp[:], scalar=1.0,
                            in1=mask_for_qc[qc][:], op0=ALU.mult, op1=ALU.add)
                        esb = sp.tile([128, S], F32, name=f"e_{tag}_{hd}_{qc}",
                                      tag="esb")
                        esum = stat.tile([128, 1], F32,
                                         name=f"es_{tag}_{hd}_{qc}", tag="esum")
                        nc.scalar.activation(esb[:], ssb[:], AF.Exp,
                                             accum_out=esum[:])
                        rinv = stat.tile([128, 1], F32,
                                         name=f"ri_{tag}_{hd}_{qc}", tag="rinv")
                        nc.vector.reciprocal(rinv[:], esum[:])
                        wn = sp.tile([128, S], BF16, name=f"wn_{tag}_{hd}_{qc}",
                                     tag="wn")
                        nc.vector.tensor_scalar_mul(wn[:], esb[:], rinv[:])
                        for kt in range(4):
                            tp = psD.tile([128, 128], BF16,
                                          name=f"wt_{tag}_{hd}_{qc}_{kt}",
                                          tag="ptw")
                            nc.tensor.transpose(tp[:],
                                                wn[:, kt * 128:(kt + 1) * 128],
                                                ident16[:])
                            nc.scalar.copy(
                                wT[sub][kt][:, qc * 128:(qc + 1) * 128], tp[:])
                avp = psA.tile([128, TC], F32, name=f"av_{tag}_{hp}", tag="pp")
                for sub in range(2):
                    hd = hp * 2 + sub
                    for kt in range(4):
                        nc.tensor.matmul(avp[sub * 64:sub * 64 + 64, :],
                                         V[kt][:, hd * 64:(hd + 1) * 64],
                                         wT[sub][kt][:],
                                         start=(kt == 0), stop=(kt == 3),
                                         tile_position=(0, sub * 64))
                at = ap_.tile([128, TC], BF16, name=f"aT_{tag}_{hp}", tag="aT")
                nc.scalar.copy(at[:], avp[:])
                aT.append(at)
            # O projection (o2-outer, stream wo column halves) + residual
            x_new = [xp.tile([128, D], F32, name=f"xn_{tag}_{t}", tag="x")
                     for t in range(2)]
            accs = [[], []]
            for o2 in range(2):
                pps = [psA.tile([128, 512], F32, name=f"po_{tag}_{o2}_{t}",
                                tag="pp") for t in range(2)]
                for kt in range(8):
                    wo_t = wsp.tile([128, 512], BF16,
                                    name=f"wo_{tag}_{o2}_{kt}", tag="ws")
                    nc.sync.dma_start(
                        out=wo_t[:],
                        in_=P["wo"][l, i, kt * 128:(kt + 1) * 128,
                                    o2 * 512:(o2 + 1) * 512])
                    for t in range(2):
                        nc.tensor.matmul(pps[t][:],
                                         aT[kt][:, t * 128:(t + 1) * 128],
                                         wo_t[:], start=(kt == 0), stop=False)
                br = bias_row(P["bo"][l, i], o2, f"o_{tag}")
                for t in range(2):
                    nc.tensor.matmul(pps[t][:], ones1[:], br[:],
                                     start=False, stop=True)
                    ac = stat.tile([128, 1], F32, name=f"ao_{tag}_{o2}_{t}",
                                   tag="acc")
                    nc.vector.scalar_tensor_tensor(
                        out=x_new[t][:, o2 * 512:(o2 + 1) * 512],
                        in0=pps[t][:], scalar=1.0,
                        in1=x_cur[t][:, o2 * 512:(o2 + 1) * 512],
                        op0=ALU.mult, op1=ALU.add, accum_out=ac[:])
                    accs[t].append(ac)
            return ln_into(x_new, accs, tag)

        # ---------------- embedding + first LN ----------------
        x_new, accs = [], []
        for t in range(2):
            xe = xp.tile([128, D], F32, name=f"xe_{t}", tag="x")
            nc.sync.dma_start(out=xe[:], in_=P["x_emb"][t * 128:(t + 1) * 128, :])
            pe = xp.tile([128, D], F32, name=f"pe_{t}", tag="x")
            nc.sync.dma_start(out=pe[:], in_=P["pos"][t * 128:(t + 1) * 128, :])
            xn = xp.tile([128, D], F32, name=f"x0_{t}", tag="x")
            ac = stat.tile([128, 1], F32, name=f"a0_{t}", tag="acc")
            nc.vector.scalar_tensor_tensor(
                out=xn[:], in0=xe[:], scalar=32.0, in1=pe[:],
                op0=ALU.mult, op1=ALU.add, accum_out=ac[:])
            x_new.append(xn)
            accs.append([ac])
        x_cur = ln_into(x_new, accs, "emb")

        # ---------------- layers ----------------
        for l in range(L):
            # ---- self attention (i=0), pairwise K/V AllGather ----
            tag = f"l{l}s"
            xT = transpose_x(x_cur, tag)
            kT_own = proj_T(load_w_rows(P["wk"][l, 0], f"wk_{tag}"), xT, TC,
                            P["bk"][l, 0], f"k_{tag}", pool=ktp, width=S)
            V_own = proj_tok(load_w_rows(P["wv"][l, 0], f"wv_{tag}"), xT, TC,
                             P["bv"][l, 0], f"v_{tag}")
            bounce_in = dram.tile([2048, TC], BF16, name=f"bin_{tag}", tag="bin")
            bounce_out = dram.tile([2 * 2048, TC], BF16, name=f"bout_{tag}",
                                   tag="bout")
            for oc in range(8):
                nc.sync.dma_start(out=bounce_in[oc * 128:(oc + 1) * 128, :],
                                  in_=kT_own[oc][:, :TC])
            for t in range(2):
                nc.sync.dma_start(
                    out=bounce_in[1024 + t * 512:1024 + (t + 1) * 512, :]
                    .rearrange("(p q) n -> p q n", p=128),
                    in_=V_own[t][:].rearrange("p (q n) -> p q n", q=4))
            nc.gpsimd.collective_compute(
                "AllGather", ALU.bypass, replica_groups=RG,
                ins=[bounce_in[:].opt()], outs=[bounce_out[:].opt()])
            kT = [ktp.tile([128, S], BF16, name=f"kTf_{tag}_{oc}", tag="ktp")
                  for oc in range(8)]
            for r in range(2):
                for oc in range(8):
                    nc.sync.dma_start(
                        out=kT[oc][:, r * TC:(r + 1) * TC],
                        in_=bounce_out[r * 2048 + oc * 128:
                                       r * 2048 + (oc + 1) * 128, :])
            V = [vp.tile([128, D], BF16, name=f"Vf_{tag}_{t}", tag="V")
                 for t in range(4)]
            for r in range(2):
                for t in range(2):
                    base = r * 2048 + 1024 + t * 512
                    nc.sync.dma_start(
                        out=V[r * 2 + t][:].rearrange("p (q n) -> p q n", q=4),
                        in_=bounce_out[base:base + 512, :]
                        .rearrange("(p q) n -> p q n", p=128))
            x_cur = attention(l, 0, xT, kT, V, causal_sb, x_cur, tag)

            # ---- cross attention blocks i=1..4 (db, con, user, enc) ----
            for i in range(1, 5):
                tag = f"l{l}c{i}"
                src = i - 1
                xT = transpose_x(x_cur, tag)
                kvT = [kvp.tile([128, S], BF16, name=f"kvT_{tag}_{kt}", tag="kvp")
                       for kt in range(8)]
                for kt in range(8):
                    nc.sync.dma_start(out=kvT[kt][:],
                                      in_=P["kvT"][src, kt * 128:(kt + 1) * 128, :])
                kT = proj_T(load_w_rows(P["wk"][l, i], f"wk_{tag}"), kvT, S,
                            P["bk"][l, i], f"k_{tag}", pool=ktp, width=S)
                V = proj_tok(load_w_rows(P["wv"][l, i], f"wv_{tag}"), kvT, S,
                             P["bv"][l, i], f"v_{tag}")
                x_cur = attention(l, i, xT, kT, V,
                                  [smask_sb[src], smask_sb[src]], x_cur, tag)

            # ---- FFN ----
            tag = f"l{l}f"
            xT = transpose_x(x_cur, tag)
            hT = []
            for fc in range(32):
                pp = psA.tile([128, 512], F32, name=f"ph_{tag}_{fc}", tag="pp")
                for kt in range(8):
                    w1t = w1p.tile([128, 128], BF16, name=f"w1_{tag}_{fc}_{kt}",
                                   tag="w1")
                    nc.sync.dma_start(
                        out=w1t[:],
                        in_=P["w1"][l, kt * 128:(kt + 1) * 128,
                                    fc * 128:(fc + 1) * 128])
                    nc.tensor.matmul(pp[:, :TC], w1t[:], xT[kt][:],
                                     start=(kt == 0), stop=(kt == 7))
                ht = htp.tile([128, TC], BF16, name=f"hT_{tag}_{fc}", tag="hT")
                bc = bias_col(P["b1"][l], fc, f"b1_{tag}")
                nc.scalar.activation(ht[:], pp[:, :TC], AF.Relu, bias=bc[:])
                hT.append(ht)
            x_new = [xp.tile([128, D], F32, name=f"xf_{tag}_{t}", tag="x")
                     for t in range(2)]
            accs = [[], []]
            for o2 in range(2):
                pps = [psA.tile([128, 512], F32, name=f"py_{tag}_{o2}_{t}",
                                tag="pp") for t in range(2)]
                for fc in range(32):
                    w2t = wsp.tile([128, 512], BF16,
                                   name=f"w2_{tag}_{o2}_{fc}", tag="ws")
                    nc.sync.dma_start(
                        out=w2t[:],
                        in_=P["w2"][l, fc * 128:(fc + 1) * 128,
                                    o2 * 512:(o2 + 1) * 512])
                    for t in range(2):
                        nc.tensor.matmul(pps[t][:],
                                         hT[fc][:, t * 128:(t + 1) * 128],
                                         w2t[:], start=(fc == 0), stop=False)
                br = bias_row(P["b2"][l], o2, f"y_{tag}")
                for t in range(2):
                    nc.tensor.matmul(pps[t][:], ones1[:], br[:],
                                     start=False, stop=True)
                    ac = stat.tile([128, 1], F32, name=f"af_{tag}_{o2}_{t}",
                                   tag="acc")
                    nc.vector.scalar_tensor_tensor(
                        out=x_new[t][:, o2 * 512:(o2 + 1) * 512],
                        in0=pps[t][:], scalar=1.0,
                        in1=x_cur[t][:, o2 * 512:(o2 + 1) * 512],
                        op0=ALU.mult, op1=ALU.add, accum_out=ac[:])
                    accs[t].append(ac)
            x_cur = ln_into(x_new, accs, tag)

        for t in range(2):
            nc.sync.dma_start(out=P["xout"][t * 128:(t + 1) * 128, :],
                              in_=x_cur[t][:])


_NC_CACHE = {}


def _get_nc():
    if "nc" not in _NC_CACHE:
        _NC_CACHE["nc"] = _build_nc()
    return _NC_CACHE["nc"]


# ---------------------------------------------------------------------------
# Host-side sharding / layout prep (indexing, casts, transposes only)
# ---------------------------------------------------------------------------

def _prep_in_maps(inputs):
    bf16 = ml_dtypes.bfloat16
    f32 = np.float32
    emb = np.asarray(inputs["sum_embeddings"], f32)
    ids = np.asarray(inputs["predict_vector"])
    posf = np.asarray(inputs["pos_emb"], f32)
    ctx = np.asarray(inputs["context_mask"])
    srcs = [np.asarray(inputs[k], f32) for k in
            ("db_graph_fc_emb", "con_graph_fc_emb", "user_graph_fc_emb",
             "encoder_latent_emb")]
    codes = [2, 1, 3, 0]  # DBPEDIA, CONCEPT, USER, PAD

    shared = dict(
        wq=np.asarray(inputs["attn_Wq"], f32).astype(bf16),
        wk=np.asarray(inputs["attn_Wk"], f32).astype(bf16),
        wv=np.asarray(inputs["attn_Wv"], f32).astype(bf16),
        wo=np.asarray(inputs["attn_Wo"], f32).astype(bf16),
        bqs=(np.asarray(inputs["attn_bq"], f32) / 8.0).astype(f32),
        bk=np.asarray(inputs["attn_bk"], f32),
        bv=np.asarray(inputs["attn_bv"], f32).astype(bf16),
        bo=np.asarray(inputs["attn_bo"], f32).astype(bf16),
        w1=np.asarray(inputs["ffn_W1"], f32).astype(bf16),
        b1=np.asarray(inputs["ffn_b1"], f32),
        w2=np.asarray(inputs["ffn_W2"], f32).astype(bf16),
        b2=np.asarray(inputs["ffn_b2"], f32).astype(bf16),
    )

    keys = np.arange(S)[None, :]
    in_maps = []
    for c in range(N_CORES):
        b, h = c // 2, c % 2
        t0 = h * TC
        causal = np.zeros((2, 128, S), f32)
        for qc in range(2):
            qglob = (t0 + qc * 128 + np.arange(128))[:, None]
            causal[qc] = np.where(keys <= qglob, 0.0, NEG)
        sm = np.zeros((4, 128, S), f32)
        for s in range(4):
            row = np.where(ctx[b] != codes[s], 0.0, NEG).astype(f32)
            sm[s] = np.broadcast_to(row[None, :], (128, S))
        kvT = np.stack([np.ascontiguousarray(srcs[s][b].T) for s in range(4)]
                       ).astype(bf16)
        in_maps.append(dict(
            x_emb=np.ascontiguousarray(emb[ids[b, t0:t0 + TC]]),
            pos=np.ascontiguousarray(posf[t0:t0 + TC]),
            causal=causal, smask=sm, kvT=kvT, **shared))
    return in_maps


def kernel(trace=False, **inputs):
    in_maps = _prep_in_maps(inputs)
    nc = _get_nc()
    res = run_bass_kernel_spmd(nc, in_maps, list(range(N_CORES)), trace=trace)
    out = np.zeros((B, T, D), np.float32)
    for c in range(N_CORES):
        b, h = c // 2, c % 2
        out[b, h * TC:(h + 1) * TC] = res.results[c]["xout"]
    if trace:
        kernel.last_res = res
    return out
